# revision 10
# baseline (speedup 1.0000x reference)
"""BlurPool3D Trainium2 kernel (8 cores; DMA roofline ~119 us/core).

Depthwise 3x3x3 separable (rank-1) blur, stride 2, pad 1 on
x[2, 64, 64, 96, 96] f32 -> y[2, 64, 32, 48, 48] f32.

Strategy (v2 -- engine-balanced against the HW cost model):
  - Shard the 128 (n, c) pairs across cores: 16 per core; 8 blocks of 2
    channels. Channels are independent in a depthwise conv -> no halo,
    no collectives. Partitions = (2 nc x 64 d) = 128; the D axis lives
    on partitions so the D-tap contraction is a matmul with a
    block-diagonal band lhsT.
  - W-pass on VectorE in fp32 (fp32 STT is 1x-mode bound, ~1.04 ns/elem
    regardless of dtype because the stride-2 reads forbid 2x packing):
    exactly 2 STTs per 49-row half-tile, zero edge ops. Trick: STT1
    computes center+right taps full-width (all reads in bounds); STT2
    adds the left tap only on w'=1..47, and w'=0 needs no left tap
    (zero pad) so it is already correct after STT1.
  - Pieces are written as fp16 -> the H+D matmul runs at 1 col/cycle
    (fp32 matmul is LOW_HIGH-emulated, ~5x slower). All 3 H taps are
    folded into 3 fp16 band matrices (H shift = stride-2 row access
    pattern on the piece tile); no DVE H-pass at all. The h zero-pad
    row is a memset row 0 of the top x tile.
  - The two 24-row output halves (g) map to PE column groups 0/1
    (tile_position (0,0)/(0,64)) -> PSUM partitions 0-63/64-127.
    Per half: own 49-row x tile (1-row halo re-DMA between halves) and
    own piece tile, so dependencies stay fine-grained.
  - ScalarE drains PSUM -> SBUF ot[128, 1152]; ONE 128-partition output
    DMA per block (partition = (g, nc, d'), 4.6 KB contiguous per
    partition), ring alternating per block.
Rel err vs fp32 reference ~1e-4 (fp16 piece rounding; the binomial /
all-ones filter weights are exact in fp16).
"""

import os
import sys

for _p in ("/opt/trn_rl_repo",):
    if _p not in sys.path and os.path.isdir(_p):
        sys.path.insert(0, _p)

import numpy as np

N, C, D, H, W = 2, 64, 64, 96, 96
DO, HO, WO = 32, 48, 48
NCORES = 8
NC_PER_CORE = (N * C) // NCORES  # 16
BLOCKS = NC_PER_CORE // 2  # 8 blocks of 2 channels each

_PROGRAM_CACHE = {}


def _rank1_factors(filt):
    """Per-channel rank-1 factorization filt[c,0] = outer(d, h, w).

    Returns (dvec, hvec, wvec) each [C, 3] with
    filt[c, 0, i, j, k] == dvec[c,i] * hvec[c,j] * wvec[c,k].
    Exact for true rank-1 filters (e.g. the binomial blur, whose entries
    are all powers of two).
    """
    dvec = np.empty((C, 3), np.float64)
    hvec = np.empty((C, 3), np.float64)
    wvec = np.empty((C, 3), np.float64)
    for c in range(C):
        T = filt[c, 0].astype(np.float64)
        idx = np.unravel_index(np.argmax(np.abs(T)), T.shape)
        i0, j0, k0 = idx
        piv = T[i0, j0, k0]
        if piv == 0.0:
            # all-zero filter
            dvec[c] = hvec[c] = wvec[c] = 0.0
            continue
        dvec[c] = T[:, j0, k0]
        hvec[c] = T[i0, :, k0] / piv
        wvec[c] = T[i0, j0, :] / piv
        recon = np.einsum("i,j,k->ijk", dvec[c], hvec[c], wvec[c])
        resid = np.abs(recon - T).max()
        if resid > 1e-6 * max(np.abs(T).max(), 1e-30):
            raise ValueError(f"filter channel {c} is not rank-1 (resid {resid})")
    return dvec, hvec, wvec


def _build_program(uniform):
    import concourse.bacc as bacc
    import concourse.mybir as mybir
    from concourse import tile

    dt = mybir.dt
    nc = bacc.Bacc("TRN2", target_bir_lowering=False, debug=False,
                   num_devices=NCORES)

    nbm = 1 if uniform else BLOCKS
    x = nc.dram_tensor("x", [NC_PER_CORE, D, H * W], dt.float32,
                       kind="ExternalInput")
    bmat = nc.dram_tensor("bmat", [128, nbm * 3 * 64], dt.float16,
                          kind="ExternalInput")
    wtaps = nc.dram_tensor("wtaps", [128, 2 * BLOCKS], dt.float32,
                           kind="ExternalInput")
    y = nc.dram_tensor("y", [NC_PER_CORE, DO, HO * WO], dt.float32,
                       kind="ExternalOutput")

    HHALF = 24
    # PSUM bank = 512 fp32/partition -> chunks of <=10 output h-rows
    chunks = [(0, 10), (10, 10), (20, 4)]

    with tile.TileContext(nc) as tc:
        with tc.tile_pool(name="const", bufs=1) as cpool, \
             tc.tile_pool(name="xp", bufs=3) as xpool, \
             tc.tile_pool(name="pp", bufs=3) as ppool, \
             tc.tile_pool(name="op", bufs=4) as opool, \
             tc.tile_pool(name="ps", bufs=6, space="PSUM") as pspool:
            bt = cpool.tile([128, nbm * 3 * 64], dt.float16)
            wt = cpool.tile([128, 2 * BLOCKS], dt.float32)
            nc.scalar.dma_start(bt[:], bmat[:])
            nc.scalar.dma_start(wt[:], wtaps[:])

            for b in range(BLOCKS):
                bcol = 0 if uniform else b * 3 * 64
                q1 = wt[:, 2 * b:2 * b + 1]
                q2 = wt[:, 2 * b + 1:2 * b + 2]
                src = x[2 * b:2 * b + 2].rearrange("a d f -> (a d) f")
                src = src.rearrange("p (h w) -> p h w", h=H)

                # Two 49-row x half-tiles; tile row j = x row (48g - 1 + j).
                # g=0 row 0 is the h zero-pad (memset); g=1 row 0 is copied
                # on-chip from the g=0 tile (x row 47) -- no halo re-DMA.
                # Each half is DMAed as 40 + 8 rows with matching W-pass
                # sub-STTs so the post-last-byte tail chain is tiny.
                # Finer DMA/STT granularity on the very last half-block
                # shortens the post-last-byte tail chain.
                fine = (b == BLOCKS - 1)
                xts, pcs = [], []
                for g, eng in ((0, nc.sync), (1, nc.scalar)):
                    xt = xpool.tile([128, 49, W], dt.float32, tag=f"x{g}")
                    if g == 0:
                        eng.dma_start(xt[:, 1:41, :], src[:, 0:40, :])
                        eng.dma_start(xt[:, 41:49, :], src[:, 40:48, :])
                    else:
                        nc.scalar.copy(xt[:, 0, :], xts[0][:, 48, :])
                        if fine:
                            eng.dma_start(xt[:, 1:21, :], src[:, 48:68, :])
                            eng.dma_start(xt[:, 21:41, :], src[:, 68:88, :])
                        else:
                            eng.dma_start(xt[:, 1:41, :], src[:, 48:88, :])
                        eng.dma_start(xt[:, 41:49, :], src[:, 88:96, :])
                    xts.append(xt)
                for g in range(2):
                    xt = xts[g]
                    pc = ppool.tile([128, 49, WO], dt.float16, tag=f"p{g}")
                    # piece[j, w'] = q1*x[2w'-1] + x[2w'] + q2*x[2w'+1]
                    # (pivot w1 folded into the band matrices).
                    # STT1 full 48 wide: center + right taps, all in bounds;
                    # w'=0 is final (left tap is zero pad).
                    # STT2 adds the left tap on w' = 1..47 (in place).
                    # g=0 row 0 (the h zero-pad) is never computed NOR read:
                    # the kh=0 matmul of chunk (0,10) skips h'=0.
                    if g and fine:
                        subs = ((0, 21), (21, 41), (41, 49))
                    else:
                        subs = ((1, 41) if g == 0 else (0, 41), (41, 49))
                    for r0, r1 in subs:
                        nc.vector.scalar_tensor_tensor(
                            pc[:, r0:r1, 0:WO],
                            xt[:, r0:r1, 1:W:2], q2,
                            xt[:, r0:r1, 0:W - 1:2],
                            mybir.AluOpType.mult, mybir.AluOpType.add)
                        nc.vector.scalar_tensor_tensor(
                            pc[:, r0:r1, 1:WO],
                            xt[:, r0:r1, 1:W - 2:2], q1, pc[:, r0:r1, 1:WO],
                            mybir.AluOpType.mult, mybir.AluOpType.add)
                    pcs.append(pc)

                # ---- fused H+D matmuls (3 H taps x 3 band diagonals) ----
                ot = opool.tile([128, HHALF * WO], dt.float32)
                pss = {}
                for h0, cnt in chunks:
                    pss[h0] = pspool.tile([128, 10 * WO], dt.float32,
                                          tag="ps", name="ps")
                for g in range(2):
                    pc = pcs[g]
                    for h0, cnt in chunks:
                        psv = pss[h0][:, :cnt * WO]
                        # g=0 chunk 0: kh=0's h'=0 tap is the h zero-pad ->
                        # skip it (emit kh order 1,2,0; kh=0 covers h'>=1
                        # via an offset PSUM dst). Piece row 0 stays unread.
                        edge = (g == 0 and h0 == 0)
                        for i, k in enumerate((1, 2, 0) if edge else (0, 1, 2)):
                            lhsT = bt[:, bcol + k * 64:bcol + (k + 1) * 64]
                            if edge and k == 0:
                                rhs = pc[:, 2:2 * cnt - 1:2, :]
                                out_ap = psv[:64, WO:]
                            else:
                                rhs = pc[:, 2 * h0 + k:
                                         2 * h0 + k + 2 * cnt - 1:2, :]
                                out_ap = psv[g * 64:, :] if g else psv[:64, :]
                            nc.tensor.matmul(
                                out_ap, lhsT, rhs,
                                start=(i == 0), stop=(i == 2),
                                tile_position=(0, 64 * g) if g else None)
                for h0, cnt in chunks:
                    nc.scalar.copy(ot[:, h0 * WO:(h0 + cnt) * WO],
                                   pss[h0][:, :cnt * WO])

                # output DMAs per (h-half, chunk): 64-partition transfers
                # spread across even (g=0) / odd (g=1) SDMA engines -> all
                # 16 busy. (A single 128-partition DMA with a 4D dram AP
                # lands every descriptor on 2 engines -- measured
                # pathology.) Per-chunk granularity keeps the final
                # transfer after the last drain small.
                for g, eng in ((0, nc.sync), (1, nc.scalar)):
                    for h0, cnt in chunks:
                        dst = y[2 * b:2 * b + 2, :,
                                g * HHALF * WO + h0 * WO:
                                g * HHALF * WO + (h0 + cnt) * WO]
                        dst = dst.rearrange("a d f -> (a d) f")
                        eng.dma_start(dst, ot[g * 64:(g + 1) * 64,
                                              h0 * WO:(h0 + cnt) * WO])
    nc.compile()
    return nc


def kernel(x, filt):
    x = np.ascontiguousarray(np.asarray(x, dtype=np.float32))
    filt = np.asarray(filt, dtype=np.float32)
    assert x.shape == (N, C, D, H, W), x.shape

    from concourse.bass_utils import run_bass_kernel_spmd

    dvec, hvec, wvec = _rank1_factors(filt)
    # W pivot = center tap w1; ratios feed the VectorE W-pass, the pivot
    # is folded into the matmul band matrices.
    w1 = wvec[:, 1].copy()
    if not (np.abs(w1) > 1e-30).all():
        raise ValueError("W center tap is zero; unsupported filter")
    q1 = wvec[:, 0] / w1
    q2 = wvec[:, 2] / w1

    uniform = bool(np.all(filt == filt[:1]))
    xr = x.reshape(N * C, D, H * W)

    in_maps = []
    for core in range(NCORES):
        chans = (np.arange(NC_PER_CORE) + core * NC_PER_CORE) % C  # local->c
        wt = np.empty((128, 2 * BLOCKS), np.float32)
        bm = np.zeros((128, (1 if uniform else BLOCKS) * 3 * 64), np.float64)
        for b in range(BLOCKS):
            for ncl in range(2):
                c = chans[2 * b + ncl]
                wt[ncl * 64:(ncl + 1) * 64, 2 * b + 0] = q1[c]
                wt[ncl * 64:(ncl + 1) * 64, 2 * b + 1] = q2[c]
                if uniform and b > 0:
                    continue
                # band matrix rows (ncl*64 + d), cols (ncl*32 + d').
                # k = 0..2: H taps; each entry dvec * hvec[k] * w1 pivot.
                for k in range(3):
                    col0 = (b * 3 + k) * 64 + ncl * 32 if not uniform \
                        else k * 64 + ncl * 32
                    for dp in range(DO):
                        for delta in range(3):
                            d = 2 * dp - 1 + delta
                            if 0 <= d < D:
                                bm[ncl * 64 + d, col0 + dp] = (
                                    dvec[c, delta] * hvec[c, k] * w1[c])
        in_maps.append({
            "x": np.ascontiguousarray(
                xr[core * NC_PER_CORE:(core + 1) * NC_PER_CORE]),
            "bmat": bm.astype(np.float16),
            "wtaps": wt,
        })

    key = ("prog2", uniform)
    if key not in _PROGRAM_CACHE:
        _PROGRAM_CACHE[key] = _build_program(uniform)
    nc = _PROGRAM_CACHE[key]

    trace = bool(int(os.environ.get("BLURPOOL_TRACE", "0")))
    kwargs = {}
    if trace and os.environ.get("BLURPOOL_TRACE_DIR"):
        kwargs["tmpdir"] = os.environ["BLURPOOL_TRACE_DIR"]
    res = run_bass_kernel_spmd(nc, in_maps, core_ids=list(range(NCORES)),
                               trace=trace, **kwargs)
    if trace:
        kernel.last_result = res

    out = np.concatenate([r["y"].reshape(NC_PER_CORE, DO, HO, WO)
                          for r in res.results], axis=0)
    return np.ascontiguousarray(out.reshape(N, C, DO, HO, WO))


# revision 13
# speedup vs baseline: 1.2165x; 1.2165x over previous
"""BlurPool3D Trainium2 kernel (8 cores; DMA roofline ~119 us/core).

Depthwise 3x3x3 separable (rank-1) blur, stride 2, pad 1 on
x[2, 64, 64, 96, 96] f32 -> y[2, 64, 32, 48, 48] f32.

Strategy (v2 -- engine-balanced against the HW cost model):
  - Shard the 128 (n, c) pairs across cores: 16 per core; 8 blocks of 2
    channels. Channels are independent in a depthwise conv -> no halo,
    no collectives. Partitions = (2 nc x 64 d) = 128; the D axis lives
    on partitions so the D-tap contraction is a matmul with a
    block-diagonal band lhsT.
  - W-pass on VectorE in fp32 (fp32 STT is 1x-mode bound, ~1.04 ns/elem
    regardless of dtype because the stride-2 reads forbid 2x packing):
    exactly 2 STTs per 49-row half-tile, zero edge ops. Trick: STT1
    computes center+right taps full-width (all reads in bounds); STT2
    adds the left tap only on w'=1..47, and w'=0 needs no left tap
    (zero pad) so it is already correct after STT1.
  - Pieces are written as fp16 -> the H+D matmul runs at 1 col/cycle
    (fp32 matmul is LOW_HIGH-emulated, ~5x slower). All 3 H taps are
    folded into 3 fp16 band matrices (H shift = stride-2 row access
    pattern on the piece tile); no DVE H-pass at all. The h zero-pad
    row is a memset row 0 of the top x tile.
  - The two 24-row output halves (g) map to PE column groups 0/1
    (tile_position (0,0)/(0,64)) -> PSUM partitions 0-63/64-127.
    Per half: own 49-row x tile (1-row halo re-DMA between halves) and
    own piece tile, so dependencies stay fine-grained.
  - ScalarE drains PSUM -> SBUF ot[128, 1152]; ONE 128-partition output
    DMA per block (partition = (g, nc, d'), 4.6 KB contiguous per
    partition), ring alternating per block.
Rel err vs fp32 reference ~1e-4 (fp16 piece rounding; the binomial /
all-ones filter weights are exact in fp16).
"""

import os
import sys

for _p in ("/opt/trn_rl_repo",):
    if _p not in sys.path and os.path.isdir(_p):
        sys.path.insert(0, _p)

import numpy as np

N, C, D, H, W = 2, 64, 64, 96, 96
DO, HO, WO = 32, 48, 48
NCORES = 8
NC_PER_CORE = (N * C) // NCORES  # 16
BLOCKS = NC_PER_CORE // 2  # 8 blocks of 2 channels each

_PROGRAM_CACHE = {}


def _rank1_factors(filt):
    """Per-channel rank-1 factorization filt[c,0] = outer(d, h, w).

    Returns (dvec, hvec, wvec) each [C, 3] with
    filt[c, 0, i, j, k] == dvec[c,i] * hvec[c,j] * wvec[c,k].
    Exact for true rank-1 filters (e.g. the binomial blur, whose entries
    are all powers of two).
    """
    dvec = np.empty((C, 3), np.float64)
    hvec = np.empty((C, 3), np.float64)
    wvec = np.empty((C, 3), np.float64)
    for c in range(C):
        T = filt[c, 0].astype(np.float64)
        idx = np.unravel_index(np.argmax(np.abs(T)), T.shape)
        i0, j0, k0 = idx
        piv = T[i0, j0, k0]
        if piv == 0.0:
            # all-zero filter
            dvec[c] = hvec[c] = wvec[c] = 0.0
            continue
        dvec[c] = T[:, j0, k0]
        hvec[c] = T[i0, :, k0] / piv
        wvec[c] = T[i0, j0, :] / piv
        recon = np.einsum("i,j,k->ijk", dvec[c], hvec[c], wvec[c])
        resid = np.abs(recon - T).max()
        if resid > 1e-6 * max(np.abs(T).max(), 1e-30):
            raise ValueError(f"filter channel {c} is not rank-1 (resid {resid})")
    return dvec, hvec, wvec


def _build_program(uniform):
    import concourse.bacc as bacc
    import concourse.mybir as mybir
    from concourse import tile

    dt = mybir.dt
    nc = bacc.Bacc("TRN2", target_bir_lowering=False, debug=False,
                   num_devices=NCORES)

    nbm = 1 if uniform else BLOCKS
    x = nc.dram_tensor("x", [NC_PER_CORE, D, H * W], dt.float32,
                       kind="ExternalInput")
    bmat = nc.dram_tensor("bmat", [128, nbm * 3 * 64], dt.float16,
                          kind="ExternalInput")
    wtaps = nc.dram_tensor("wtaps", [128, 2 * BLOCKS], dt.float32,
                           kind="ExternalInput")
    y = nc.dram_tensor("y", [NC_PER_CORE, DO, HO * WO], dt.float32,
                       kind="ExternalOutput")

    HHALF = 24
    # PSUM bank = 512 fp32/partition -> chunks of <=10 output h-rows
    chunks = [(0, 10), (10, 10), (20, 4)]

    with tile.TileContext(nc) as tc:
        with tc.tile_pool(name="const", bufs=1) as cpool, \
             tc.tile_pool(name="xp", bufs=4) as xpool, \
             tc.tile_pool(name="pp", bufs=3) as ppool, \
             tc.tile_pool(name="op", bufs=4) as opool, \
             tc.tile_pool(name="ps", bufs=6, space="PSUM") as pspool:
            bt = cpool.tile([128, nbm * 3 * 64], dt.float16)
            wt = cpool.tile([128, 2 * BLOCKS], dt.float32)
            nc.scalar.dma_start(bt[:], bmat[:])
            nc.scalar.dma_start(wt[:], wtaps[:])

            for b in range(BLOCKS):
                bcol = 0 if uniform else b * 3 * 64
                q1 = wt[:, 2 * b:2 * b + 1]
                q2 = wt[:, 2 * b + 1:2 * b + 2]
                src = x[2 * b:2 * b + 2].rearrange("a d f -> (a d) f")
                src = src.rearrange("p (h w) -> p h w", h=H)

                # Two 49-row x half-tiles; tile row j = x row (48g - 1 + j).
                # g=0 row 0 is the h zero-pad (memset); g=1 row 0 is copied
                # on-chip from the g=0 tile (x row 47) -- no halo re-DMA.
                # Each half is DMAed as 40 + 8 rows with matching W-pass
                # sub-STTs so the post-last-byte tail chain is tiny.
                # Finer DMA/STT granularity on the very last half-block
                # shortens the post-last-byte tail chain.
                fine = (b == BLOCKS - 1)
                xts, pcs = [], []
                for g, eng in ((0, nc.sync), (1, nc.scalar)):
                    xt = xpool.tile([128, 49, W], dt.float32, tag=f"x{g}")
                    if g == 0:
                        if fine:
                            eng.dma_start(xt[:, 1:41, :], src[:, 0:40, :])
                            eng.dma_start(xt[:, 41:49, :], src[:, 40:48, :])
                        else:
                            eng.dma_start(xt[:, 1:49, :], src[:, 0:48, :])
                    else:
                        nc.scalar.copy(xt[:, 0, :], xts[0][:, 48, :])
                        if fine:
                            eng.dma_start(xt[:, 1:21, :], src[:, 48:68, :])
                            eng.dma_start(xt[:, 21:41, :], src[:, 68:88, :])
                            eng.dma_start(xt[:, 41:49, :], src[:, 88:96, :])
                        else:
                            eng.dma_start(xt[:, 1:49, :], src[:, 48:96, :])
                    xts.append(xt)
                for g in range(2):
                    xt = xts[g]
                    pc = ppool.tile([128, 49, WO], dt.float16, tag=f"p{g}")
                    # piece[j, w'] = q1*x[2w'-1] + x[2w'] + q2*x[2w'+1]
                    # (pivot w1 folded into the band matrices).
                    # STT1 full 48 wide: center + right taps, all in bounds;
                    # w'=0 is final (left tap is zero pad).
                    # STT2 adds the left tap on w' = 1..47 (in place).
                    # g=0 row 0 (the h zero-pad) is never computed NOR read:
                    # the kh=0 matmul of chunk (0,10) skips h'=0.
                    if not fine:
                        subs = ((1, 49) if g == 0 else (0, 49),)
                    elif g:
                        subs = ((0, 21), (21, 41), (41, 49))
                    else:
                        subs = ((1, 41), (41, 49))
                    for r0, r1 in subs:
                        nc.vector.scalar_tensor_tensor(
                            pc[:, r0:r1, 0:WO],
                            xt[:, r0:r1, 1:W:2], q2,
                            xt[:, r0:r1, 0:W - 1:2],
                            mybir.AluOpType.mult, mybir.AluOpType.add)
                        nc.vector.scalar_tensor_tensor(
                            pc[:, r0:r1, 1:WO],
                            xt[:, r0:r1, 1:W - 2:2], q1, pc[:, r0:r1, 1:WO],
                            mybir.AluOpType.mult, mybir.AluOpType.add)
                    pcs.append(pc)

                # ---- fused H+D matmuls (3 H taps x 3 band diagonals) ----
                ot = opool.tile([128, HHALF * WO], dt.float32)
                pss = {}
                for h0, cnt in chunks:
                    pss[h0] = pspool.tile([128, 10 * WO], dt.float32,
                                          tag="ps", name="ps")
                for g in range(2):
                    pc = pcs[g]
                    for h0, cnt in chunks:
                        psv = pss[h0][:, :cnt * WO]
                        # g=0 chunk 0: kh=0's h'=0 tap is the h zero-pad ->
                        # skip it (emit kh order 1,2,0; kh=0 covers h'>=1
                        # via an offset PSUM dst). Piece row 0 stays unread.
                        edge = (g == 0 and h0 == 0)
                        for i, k in enumerate((1, 2, 0) if edge else (0, 1, 2)):
                            lhsT = bt[:, bcol + k * 64:bcol + (k + 1) * 64]
                            if edge and k == 0:
                                rhs = pc[:, 2:2 * cnt - 1:2, :]
                                out_ap = psv[:64, WO:]
                            else:
                                rhs = pc[:, 2 * h0 + k:
                                         2 * h0 + k + 2 * cnt - 1:2, :]
                                out_ap = psv[g * 64:, :] if g else psv[:64, :]
                            nc.tensor.matmul(
                                out_ap, lhsT, rhs,
                                start=(i == 0), stop=(i == 2),
                                tile_position=(0, 64 * g) if g else None)
                for h0, cnt in chunks:
                    nc.scalar.copy(ot[:, h0 * WO:(h0 + cnt) * WO],
                                   pss[h0][:, :cnt * WO])

                # output DMAs per (h-half, chunk): 64-partition transfers
                # spread across even (g=0) / odd (g=1) SDMA engines -> all
                # 16 busy. (A single 128-partition DMA with a 4D dram AP
                # lands every descriptor on 2 engines -- measured
                # pathology.) Per-chunk granularity keeps the final
                # transfer after the last drain small.
                for g, eng in ((0, nc.sync), (1, nc.scalar)):
                    for h0, cnt in chunks:
                        dst = y[2 * b:2 * b + 2, :,
                                g * HHALF * WO + h0 * WO:
                                g * HHALF * WO + (h0 + cnt) * WO]
                        dst = dst.rearrange("a d f -> (a d) f")
                        eng.dma_start(dst, ot[g * 64:(g + 1) * 64,
                                              h0 * WO:(h0 + cnt) * WO])
    nc.compile()
    return nc


def kernel(x, filt):
    x = np.ascontiguousarray(np.asarray(x, dtype=np.float32))
    filt = np.asarray(filt, dtype=np.float32)
    assert x.shape == (N, C, D, H, W), x.shape

    from concourse.bass_utils import run_bass_kernel_spmd

    dvec, hvec, wvec = _rank1_factors(filt)
    # W pivot = center tap w1; ratios feed the VectorE W-pass, the pivot
    # is folded into the matmul band matrices.
    w1 = wvec[:, 1].copy()
    if not (np.abs(w1) > 1e-30).all():
        raise ValueError("W center tap is zero; unsupported filter")
    q1 = wvec[:, 0] / w1
    q2 = wvec[:, 2] / w1

    uniform = bool(np.all(filt == filt[:1]))
    xr = x.reshape(N * C, D, H * W)

    in_maps = []
    for core in range(NCORES):
        chans = (np.arange(NC_PER_CORE) + core * NC_PER_CORE) % C  # local->c
        wt = np.empty((128, 2 * BLOCKS), np.float32)
        bm = np.zeros((128, (1 if uniform else BLOCKS) * 3 * 64), np.float64)
        for b in range(BLOCKS):
            for ncl in range(2):
                c = chans[2 * b + ncl]
                wt[ncl * 64:(ncl + 1) * 64, 2 * b + 0] = q1[c]
                wt[ncl * 64:(ncl + 1) * 64, 2 * b + 1] = q2[c]
                if uniform and b > 0:
                    continue
                # band matrix rows (ncl*64 + d), cols (ncl*32 + d').
                # k = 0..2: H taps; each entry dvec * hvec[k] * w1 pivot.
                for k in range(3):
                    col0 = (b * 3 + k) * 64 + ncl * 32 if not uniform \
                        else k * 64 + ncl * 32
                    for dp in range(DO):
                        for delta in range(3):
                            d = 2 * dp - 1 + delta
                            if 0 <= d < D:
                                bm[ncl * 64 + d, col0 + dp] = (
                                    dvec[c, delta] * hvec[c, k] * w1[c])
        in_maps.append({
            "x": np.ascontiguousarray(
                xr[core * NC_PER_CORE:(core + 1) * NC_PER_CORE]),
            "bmat": bm.astype(np.float16),
            "wtaps": wt,
        })

    key = ("prog2", uniform)
    if key not in _PROGRAM_CACHE:
        _PROGRAM_CACHE[key] = _build_program(uniform)
    nc = _PROGRAM_CACHE[key]

    trace = bool(int(os.environ.get("BLURPOOL_TRACE", "0")))
    kwargs = {}
    if trace and os.environ.get("BLURPOOL_TRACE_DIR"):
        kwargs["tmpdir"] = os.environ["BLURPOOL_TRACE_DIR"]
    res = run_bass_kernel_spmd(nc, in_maps, core_ids=list(range(NCORES)),
                               trace=trace, **kwargs)
    if trace:
        kernel.last_result = res

    out = np.concatenate([r["y"].reshape(NC_PER_CORE, DO, HO, WO)
                          for r in res.results], axis=0)
    return np.ascontiguousarray(out.reshape(N, C, DO, HO, WO))
